# revision 5
# baseline (speedup 1.0000x reference)
"""VQ codebook encoding kernel for Trainium2 (8 NeuronCores, data-parallel over B).

Per core (one batch element):
  X [D=128, N=4096], codewords C [K=32, D=128], scale s [K=32]
  dist2[n,k] = x_sq[n] - 2*cross[n,k] + c_sq[k]
  A = softmax_k(dist2 * s)            (no max-subtraction needed: |logit| <= ~50)
  E[k,d] = sum_n A[n,k] * XT[n,d] - (sum_n A[n,k]) * C[k,d]

Dataflow: 32 column-blocks of 128. Per block the PE transposes X_b (PSUM),
computes crossS_b = X_b^T @ (-2 s C)^T directly in [n,k] layout, and later
accumulates E (and the A-colsum in column 128) with one [32,129] accumulating
matmul chain. ScalarE copies XT PSUM->SBUF; DVE does x_sq via
tensor_tensor_reduce, logits assembly with broadcast APs, rowsum + reciprocal +
normalize.
"""

import os
import numpy as np

B, D, K, N = 8, 128, 32, 4096
NBLK = 32          # column blocks of 128
GRP = 16           # blocks per softmax group
NGRP = NBLK // GRP

_cache = {}


def _build_program():
    import concourse.bacc as bacc
    import concourse.bass as bass
    import concourse.tile as tile
    from concourse import mybir
    from concourse.masks import make_identity

    fp32 = mybir.dt.float32
    Alu = mybir.AluOpType
    Act = mybir.ActivationFunctionType

    nc = bacc.Bacc(
        "TRN2",
        target_bir_lowering=False,
        debug=False,
        num_devices=8,
    )

    x_dram = nc.dram_tensor("X", [D, N], fp32, kind="ExternalInput").ap()
    c_dram = nc.dram_tensor("codewords", [K, D], fp32, kind="ExternalInput").ap()
    s_dram = nc.dram_tensor("scale", [K], fp32, kind="ExternalInput").ap()
    e_dram = nc.dram_tensor("E", [K, D], fp32, kind="ExternalOutput").ap()

    def bcast_inner(ap, n):
        # [P, m] -> [P, m, n] with the inner dim broadcast (step 0)
        return bass.AP(tensor=ap.tensor, offset=ap.offset, ap=list(ap.ap) + [[0, n]])

    def bcast_mid(ap, n):
        # [P, m] -> [P, n, m] with the middle dim broadcast (step 0)
        a = list(ap.ap)
        return bass.AP(tensor=ap.tensor, offset=ap.offset, ap=[a[0], [0, n]] + a[1:])

    with tile.TileContext(nc) as tc:
        import contextlib

        ctx = contextlib.ExitStack()
        with ctx:
            sing = ctx.enter_context(tc.tile_pool(name="sing", bufs=1))
            xtq_pool = ctx.enter_context(
                tc.tile_pool(name="xtq", bufs=3, space="PSUM")
            )
            cross_pool = ctx.enter_context(
                tc.tile_pool(name="crossp", bufs=2, space="PSUM")
            )
            e_pool = ctx.enter_context(tc.tile_pool(name="ep", bufs=1, space="PSUM"))
            setup_pool = ctx.enter_context(
                tc.tile_pool(name="setupp", bufs=1, space="PSUM")
            )
            dram_pool = ctx.enter_context(
                tc.tile_pool(name="dramp", bufs=1, space="DRAM")
            )

            # ---------------- persistent SBUF tensors ----------------
            x_sb = sing.tile([D, N], fp32)            # X natural [d, n]
            xt_sb = sing.tile([128, NBLK, 129], fp32)  # XT blocks + ones col
            a_sb = sing.tile([128, NBLK, K], fp32)     # softmax output A
            l_sb = sing.tile([128, NBLK, K], fp32)     # logits
            u_sb = sing.tile([128, NBLK, K], fp32)     # exp(logits)
            p1_sb = sing.tile([128, NBLK, K], fp32)    # s*x_sq outer
            t2_sb = sing.tile([128, NBLK, K], fp32)    # crossS + cs
            xsq_sb = sing.tile([128, NBLK], fp32)
            den_sb = sing.tile([128, NGRP * GRP // GRP * GRP], fp32)  # [128, 32]
            rec_sb = sing.tile([128, NBLK], fp32)
            junk_sb = sing.tile([128, 128], fp32)      # TTR product dump

            ident_sb = sing.tile([128, 128], fp32)
            c_sb = sing.tile([K, D], fp32)
            c2s_sb = sing.tile([K, D], fp32)
            c2st_sb = sing.tile([D, K], fp32)
            s_col = sing.tile([K, 1], fp32)
            csq_col = sing.tile([K, 1], fp32)
            cs_col = sing.tile([K, 1], fp32)
            s_b128 = sing.tile([128, K], fp32)
            cs_b128 = sing.tile([128, K], fp32)
            csq_junk = sing.tile([K, D], fp32)
            s_mcol = sing.tile([K, 1], fp32)          # S = colsum(A), copied out
            ct_tmp = sing.tile([K, D], fp32)          # S * C
            e_out = sing.tile([K, D], fp32)

            cs_dram = dram_pool.tile([K], fp32)

            # ---------------- setup ----------------
            make_identity(nc, ident_sb[:])
            nc.vector.memset(xt_sb[:, :, 128:129], 1.0)

            nc.sync.dma_start(out=c_sb[:], in_=c_dram)
            nc.sync.dma_start(
                out=s_col[:], in_=s_dram.rearrange("(p o) -> p o", o=1)
            )
            # broadcast scale across partitions straight from DRAM
            nc.sync.dma_start(
                out=s_b128[:],
                in_=bass.AP(tensor=s_dram.tensor, offset=s_dram.offset,
                            ap=[[0, 128], [1, K]]),
            )

            # c_sq[k] = sum_d C[k,d]^2 ; cs = c_sq * s
            nc.vector.scalar_tensor_tensor(
                out=csq_junk[:],
                in0=c_sb[:],
                scalar=1.0,
                in1=c_sb[:],
                op0=Alu.mult,
                op1=Alu.mult,
                accum_out=csq_col[:],
            )
            nc.vector.tensor_mul(cs_col[:], csq_col[:], s_col[:])
            # round-trip through DRAM to broadcast cs over partitions
            nc.sync.dma_start(
                out=cs_dram[:], in_=cs_col[:].rearrange("p o -> (p o)")
            )
            nc.sync.dma_start(
                out=cs_b128[:],
                in_=bass.AP(tensor=cs_dram[:].tensor, offset=cs_dram[:].offset,
                            ap=[[0, 128], [1, K]]),
            )

            # C2s = -2 * s * C  (fold -2*scale into the codebook)
            nc.vector.tensor_scalar(
                out=c2s_sb[:],
                in0=c_sb[:],
                scalar1=s_col[:],
                scalar2=-2.0,
                op0=Alu.mult,
                op1=Alu.mult,
            )
            # C2sT = transpose(C2s) via PE
            c2st_psum = setup_pool.tile([D, K], fp32)
            nc.tensor.transpose(c2st_psum[:], c2s_sb[:], ident_sb[:K, :K])
            nc.scalar.copy(c2st_sb[:], c2st_psum[:])

            # ---------------- X load ----------------
            for ch in range(8):
                nc.sync.dma_start(
                    out=x_sb[:, ch * 512:(ch + 1) * 512],
                    in_=x_dram[:, ch * 512:(ch + 1) * 512],
                )

            e_psum = e_pool.tile([K, 129], fp32)

            cross_psum = [None, None]

            def emit_front(b):
                """transpose + cross matmul + XT copyback + x_sq for block b"""
                g, j = b // GRP, b % GRP
                q, r = b // 4, b % 4
                if j == 0:
                    cross_psum[g] = cross_pool.tile([128, GRP, K], fp32, name=f'cross_{g}', tag='cross')
                xb = x_sb[:, b * 128:(b + 1) * 128]
                if r == 0:
                    emit_front.xtq = xtq_pool.tile([128, 4, 128], fp32, name=f'xtq_{b}', tag='xtq')
                xtq = emit_front.xtq
                nc.tensor.transpose(xtq[:, r, :], xb, ident_sb[:])
                nc.tensor.matmul(
                    cross_psum[g][:, j, :], lhsT=xb, rhs=c2st_sb[:],
                    start=True, stop=True,
                )
                if r == 3:
                    # copy 4 transposed blocks PSUM -> SBUF in one ScalarE op
                    nc.scalar.copy(
                        xt_sb[:, q * 4:q * 4 + 4, :128], xtq[:, :, :]
                    )
                    for bb in range(q * 4, q * 4 + 4):
                        nc.vector.scalar_tensor_tensor(
                            out=junk_sb[:],
                            in0=xt_sb[:, bb, :128],
                            scalar=1.0,
                            in1=xt_sb[:, bb, :128],
                            op0=Alu.mult,
                            op1=Alu.mult,
                            accum_out=xsq_sb[:, bb:bb + 1],
                        )

            def emit_softmax(g):
                """softmax for group g (GRP blocks)"""
                sl = slice(g * GRP, (g + 1) * GRP)
                # P1 = outer(x_sq, s)
                nc.vector.tensor_tensor(
                    out=p1_sb[:, sl, :],
                    in0=bcast_inner(xsq_sb[:, sl], K),
                    in1=bcast_mid(s_b128[:], GRP),
                    op=Alu.mult,
                )
                # T2 = crossS + cs
                nc.vector.tensor_tensor(
                    out=t2_sb[:, sl, :],
                    in0=cross_psum[g][:],
                    in1=bcast_mid(cs_b128[:], GRP),
                    op=Alu.add,
                )
                # L = P1 + T2
                nc.vector.tensor_tensor(
                    out=l_sb[:, sl, :],
                    in0=p1_sb[:, sl, :],
                    in1=t2_sb[:, sl, :],
                    op=Alu.add,
                )
                # U = exp(L)
                nc.scalar.activation(
                    u_sb[:, sl, :].rearrange("p a b -> p (a b)"),
                    l_sb[:, sl, :].rearrange("p a b -> p (a b)"),
                    Act.Exp,
                )
                # den = sum_k U ; rec = 1/den ; A = U * rec
                nc.vector.reduce_sum(
                    den_sb[:, sl], u_sb[:, sl, :], axis=mybir.AxisListType.X
                )
                nc.vector.reciprocal(rec_sb[:, sl], den_sb[:, sl])
                nc.vector.tensor_tensor(
                    out=a_sb[:, sl, :],
                    in0=u_sb[:, sl, :],
                    in1=bcast_inner(rec_sb[:, sl], K),
                    op=Alu.mult,
                )

            def emit_e(b):
                nc.tensor.matmul(
                    e_psum[:],
                    lhsT=a_sb[:, b, :],
                    rhs=xt_sb[:, b, :],
                    start=(b == 0),
                    stop=(b == NBLK - 1),
                )

            for b in range(GRP):
                emit_front(b)
            emit_softmax(0)
            for b in range(GRP, NBLK):
                emit_front(b)
                emit_e(b - GRP)
            emit_softmax(1)
            for b in range(GRP, NBLK):
                emit_e(b)

            # E = e_psum[:, :128] - S * C  via  (C * -S) + e_psum
            nc.scalar.mul(s_mcol[:], e_psum[:, 128:129], -1.0)
            nc.vector.scalar_tensor_tensor(
                out=e_out[:],
                in0=c_sb[:],
                scalar=s_mcol[:],
                in1=e_psum[:, :128],
                op0=Alu.mult,
                op1=Alu.add,
            )
            nc.sync.dma_start(out=e_dram, in_=e_out[:])

    nc.compile()
    return nc


def _get_program():
    if "nc" not in _cache:
        _cache["nc"] = _build_program()
    return _cache["nc"]


def kernel(X, codewords, scale):
    from concourse.bass_utils import run_bass_kernel_spmd

    X = np.ascontiguousarray(np.asarray(X, dtype=np.float32))
    codewords = np.ascontiguousarray(np.asarray(codewords, dtype=np.float32))
    scale = np.ascontiguousarray(np.asarray(scale, dtype=np.float32))

    nc = _get_program()
    xs = X.reshape(B, D, N)
    in_maps = [
        {"X": xs[i], "codewords": codewords, "scale": scale} for i in range(B)
    ]
    res = run_bass_kernel_spmd(nc, in_maps, core_ids=list(range(B)))
    out = np.stack([res.results[i]["E"] for i in range(B)])
    return out.astype(np.float32)


# revision 20
# speedup vs baseline: 15182.2596x; 15182.2596x over previous
"""VQ codebook encoding kernel for Trainium2 (8 NeuronCores, data-parallel over B).

Per core (one batch element):
  X [D=128, N=4096], codewords C [K=32, D=128], scale s [K=32]
  dist2[n,k] = x_sq[n] - 2*cross[n,k] + c_sq[k]
  A = softmax_k(dist2 * s)            (no max-subtraction needed: |logit| <= ~50)
  E[k,d] = sum_n A[n,k] * XT[n,d] - (sum_n A[n,k]) * C[k,d]

Dataflow: 32 column-blocks of 128, 4 softmax groups of 8 blocks. Per block the
PE transposes X_b (PSUM) and computes crossS_b = X_b^T @ (-2 s C)^T directly in
[n,k] layout; E (plus the A-colsum in column 128) accumulates with one [32,129]
matmul chain. ScalarE copies XT PSUM->SBUF (4-block chunks); DVE does x_sq via
scalar_tensor_tensor accum, PSUM-side logit add, rowsum + reciprocal; GpSimd
does the SBUF-only logit muls/adds and the A normalize.
"""

import os
import numpy as np

B, D, K, N = 8, 128, 32, 4096
NBLK = 32          # column blocks of 128
GRP = 8            # blocks per softmax group
NGRP = NBLK // GRP

_cache = {}


def _build_program():
    import concourse.bacc as bacc
    import concourse.bass as bass
    import concourse.tile as tile
    from concourse import mybir
    from concourse.masks import make_identity

    fp32 = mybir.dt.float32
    Alu = mybir.AluOpType
    Act = mybir.ActivationFunctionType

    nc = bacc.Bacc(
        "TRN2",
        target_bir_lowering=False,
        debug=False,
        num_devices=8,
    )

    x_dram = nc.dram_tensor("X", [D, N], fp32, kind="ExternalInput").ap()
    c_dram = nc.dram_tensor("codewords", [K, D], fp32, kind="ExternalInput").ap()
    s_dram = nc.dram_tensor("scale", [K], fp32, kind="ExternalInput").ap()
    e_dram = nc.dram_tensor("E", [K, D], fp32, kind="ExternalOutput").ap()

    def bcast_inner(ap, n):
        # [P, m] -> [P, m, n] with the inner dim broadcast (step 0)
        return bass.AP(tensor=ap.tensor, offset=ap.offset, ap=list(ap.ap) + [[0, n]])

    def bcast_mid(ap, n):
        # [P, m] -> [P, n, m] with the middle dim broadcast (step 0)
        a = list(ap.ap)
        return bass.AP(tensor=ap.tensor, offset=ap.offset, ap=[a[0], [0, n]] + a[1:])

    with tile.TileContext(nc) as tc:
        import contextlib

        ctx = contextlib.ExitStack()
        with ctx:
            sing = ctx.enter_context(tc.tile_pool(name="sing", bufs=1))
            xtq_pool = ctx.enter_context(
                tc.tile_pool(name="xtq", bufs=4, space="PSUM")
            )
            cross_pool = ctx.enter_context(
                tc.tile_pool(name="crossp", bufs=2, space="PSUM")
            )
            e_pool = ctx.enter_context(tc.tile_pool(name="ep", bufs=1, space="PSUM"))
            setup_pool = ctx.enter_context(
                tc.tile_pool(name="setupp", bufs=1, space="PSUM")
            )
            dram_pool = ctx.enter_context(
                tc.tile_pool(name="dramp", bufs=1, space="DRAM")
            )

            # ---------------- persistent SBUF tensors ----------------
            x_sb = sing.tile([D, N], fp32)             # X natural [d, n]
            xt_sb = sing.tile([128, NBLK, 129], fp32)  # XT blocks + ones col
            a_sb = sing.tile([128, NBLK, K], fp32)     # softmax output A
            l_sb = sing.tile([128, NBLK, K], fp32)     # logits
            u_sb = sing.tile([128, NBLK, K], fp32)     # exp(logits)
            p1_sb = sing.tile([128, NBLK, K], fp32)    # s*x_sq outer
            t2_sb = sing.tile([128, NBLK, K], fp32)    # crossS + cs
            xsq_sb = sing.tile([128, NBLK], fp32)
            den_sb = sing.tile([128, NBLK], fp32)
            rec_sb = sing.tile([128, NBLK], fp32)
            junk_sb = sing.tile([128, 128], fp32)      # STT product dump (DVE)
            junk2_sb = sing.tile([128, 128], fp32)     # Square dump (ScalarE)

            ident_sb = sing.tile([128, 128], fp32)
            c_sb = sing.tile([K, D], fp32)
            c2s_sb = sing.tile([K, D], fp32)
            c2st_sb = sing.tile([D, K], fp32)
            s_col = sing.tile([K, 1], fp32)
            csq_col = sing.tile([K, 1], fp32)
            cs_col = sing.tile([K, 1], fp32)
            s_b128 = sing.tile([128, K], fp32)
            cs_b128 = sing.tile([128, K], fp32)
            csq_junk = sing.tile([K, D], fp32)
            s_mcol = sing.tile([K, 1], fp32)           # -S, S = colsum(A)
            e_out = sing.tile([K, D], fp32)

            cs_dram = dram_pool.tile([K], fp32)

            # identity + ones column first: Pool must finish these before its
            # SWDGE X loads, and the first PE transpose needs the identity.
            make_identity(nc, ident_sb[:])
            nc.vector.memset(xt_sb[:, :, 128:129], 1.0)

            # ---------------- DMA in ----------------
            # X split over two DGE queues (HWDGE via sync, SWDGE via gpsimd) so
            # the per-DMA queue-serialization overlaps; small first chunk so the
            # PE can start transposing early.
            def xload(eng, lo, hi):
                eng.dma_start(out=x_sb[:, lo:hi], in_=x_dram[:, lo:hi])

            xload(nc.sync, 0, 256)
            nc.sync.dma_start(out=c_sb[:], in_=c_dram)
            nc.sync.dma_start(
                out=s_col[:], in_=s_dram.rearrange("(p o) -> p o", o=1)
            )
            # broadcast scale across partitions straight from DRAM
            nc.sync.dma_start(
                out=s_b128[:],
                in_=bass.AP(tensor=s_dram.tensor, offset=s_dram.offset,
                            ap=[[0, 128], [1, K]]),
            )
            xload(nc.gpsimd, 2048, 3072)
            xload(nc.gpsimd, 3072, 4096)
            xload(nc.sync, 256, 1024)
            xload(nc.sync, 1024, 2048)

            # ---------------- setup ----------------
            # C2s = -2 * s * C first: the cross matmuls gate on C2sT, while
            # cs_b128 is only needed by the first softmax (~8us in).
            nc.vector.tensor_scalar(
                out=c2s_sb[:],
                in0=c_sb[:],
                scalar1=s_col[:],
                scalar2=-2.0,
                op0=Alu.mult,
                op1=Alu.mult,
            )
            # C2sT = transpose(C2s) via PE
            c2st_psum = setup_pool.tile([D, K], fp32)
            nc.tensor.transpose(c2st_psum[:], c2s_sb[:], ident_sb[:K, :K])
            # DVE copy: the first ScalarE op pays the ~2.7us ACT table load,
            # which must stay off the cross-matmul critical path.
            nc.vector.tensor_copy(c2st_sb[:], c2st_psum[:])

            # c_sq[k] = sum_d C[k,d]^2 ; cs = c_sq * s
            nc.vector.scalar_tensor_tensor(
                out=csq_junk[:],
                in0=c_sb[:],
                scalar=1.0,
                in1=c_sb[:],
                op0=Alu.mult,
                op1=Alu.mult,
                accum_out=csq_col[:],
            )
            nc.vector.tensor_mul(cs_col[:], csq_col[:], s_col[:])
            # round-trip through DRAM to broadcast cs over partitions
            nc.sync.dma_start(
                out=cs_dram[:], in_=cs_col[:].rearrange("p o -> (p o)")
            )
            nc.sync.dma_start(
                out=cs_b128[:],
                in_=bass.AP(tensor=cs_dram[:].tensor, offset=cs_dram[:].offset,
                            ap=[[0, 128], [1, K]]),
            )

            e_psum = e_pool.tile([K, 129], fp32)

            cross_tiles = [None, None]   # one PSUM bank holds two groups

            def cross_view(g):
                return cross_tiles[g // 2][:, (g % 2) * GRP:(g % 2 + 1) * GRP, :]

            def emit_front(b):
                """transpose + cross matmul + XT copyback + x_sq for block b"""
                g, j = b // GRP, b % GRP
                q, r = b // 4, b % 4
                if g % 2 == 0 and j == 0:
                    cross_tiles[g // 2] = cross_pool.tile(
                        [128, 2 * GRP, K], fp32, name=f'cross_{g}', tag='cross'
                    )
                xb = x_sb[:, b * 128:(b + 1) * 128]
                if r == 0:
                    emit_front.xtq = xtq_pool.tile(
                        [128, 4, 128], fp32, name=f'xtq_{b}', tag='xtq'
                    )
                xtq = emit_front.xtq
                nc.tensor.transpose(xtq[:, r, :], xb, ident_sb[:])
                nc.tensor.matmul(
                    cross_view(g)[:, j, :], lhsT=xb, rhs=c2st_sb[:],
                    start=True, stop=True,
                )
                if r == 3:
                    # copy 4 transposed blocks PSUM -> SBUF in one ScalarE op
                    nc.scalar.copy(
                        xt_sb[:, q * 4:q * 4 + 4, :128], xtq[:, :, :]
                    )
                    for bb in range(q * 4, q * 4 + 4):
                        if bb % 4 == 1:
                            # offload one x_sq per quad to ScalarE (PSUM src)
                            nc.scalar.activation(
                                junk2_sb[:],
                                xtq[:, bb % 4, :],
                                Act.Square,
                                accum_out=xsq_sb[:, bb:bb + 1],
                            )
                        else:
                            nc.vector.scalar_tensor_tensor(
                                out=junk_sb[:],
                                in0=xt_sb[:, bb, :128],
                                scalar=1.0,
                                in1=xt_sb[:, bb, :128],
                                op0=Alu.mult,
                                op1=Alu.mult,
                                accum_out=xsq_sb[:, bb:bb + 1],
                            )

            def emit_softmax(g):
                """softmax for group g (GRP blocks)"""
                sl = slice(g * GRP, (g + 1) * GRP)
                # P1 = outer(x_sq, s)          [GpSimd]
                nc.gpsimd.tensor_tensor(
                    out=p1_sb[:, sl, :],
                    in0=bcast_inner(xsq_sb[:, sl], K),
                    in1=bcast_mid(s_b128[:], GRP),
                    op=Alu.mult,
                )
                # T2 = crossS + cs             [DVE, PSUM side]
                nc.vector.tensor_tensor(
                    out=t2_sb[:, sl, :],
                    in0=cross_view(g),
                    in1=bcast_mid(cs_b128[:], GRP),
                    op=Alu.add,
                )
                # L = P1 + T2                  [DVE]
                nc.vector.tensor_tensor(
                    out=l_sb[:, sl, :],
                    in0=p1_sb[:, sl, :],
                    in1=t2_sb[:, sl, :],
                    op=Alu.add,
                )
                # U = exp(L)                   [ScalarE]
                nc.scalar.activation(
                    u_sb[:, sl, :].rearrange("p a b -> p (a b)"),
                    l_sb[:, sl, :].rearrange("p a b -> p (a b)"),
                    Act.Exp,
                )
                # den = sum_k U ; rec = 1/den  [DVE]
                nc.vector.reduce_sum(
                    den_sb[:, sl], u_sb[:, sl, :], axis=mybir.AxisListType.X
                )
                nc.vector.reciprocal(rec_sb[:, sl], den_sb[:, sl])
                # A = U * rec                  [DVE]
                nc.vector.tensor_tensor(
                    out=a_sb[:, sl, :],
                    in0=u_sb[:, sl, :],
                    in1=bcast_inner(rec_sb[:, sl], K),
                    op=Alu.mult,
                )

            def emit_e(b):
                nc.tensor.matmul(
                    e_psum[:],
                    lhsT=a_sb[:, b, :],
                    rhs=xt_sb[:, b, :],
                    start=(b == 0),
                    stop=(b == NBLK - 1),
                )

            # pipeline: front(g) ... softmax(g) ... E(g) interleaved with front(g+1)
            for g in range(NGRP):
                for b in range(g * GRP, (g + 1) * GRP):
                    emit_front(b)
                emit_softmax(g)
                if g >= 1:
                    for b in range((g - 1) * GRP, g * GRP):
                        emit_e(b)
            for b in range((NGRP - 1) * GRP, NBLK):
                emit_e(b)

            # E = e_psum[:, :128] - S * C  via  (C * -S) + e_psum
            nc.scalar.mul(s_mcol[:], e_psum[:, 128:129], -1.0)
            nc.vector.scalar_tensor_tensor(
                out=e_out[:],
                in0=c_sb[:],
                scalar=s_mcol[:],
                in1=e_psum[:, :128],
                op0=Alu.mult,
                op1=Alu.add,
            )
            nc.sync.dma_start(out=e_dram, in_=e_out[:])

    nc.compile()
    return nc


def _get_program():
    if "nc" not in _cache:
        _cache["nc"] = _build_program()
    return _cache["nc"]


def kernel(X, codewords, scale):
    from concourse.bass_utils import run_bass_kernel_spmd

    X = np.ascontiguousarray(np.asarray(X, dtype=np.float32))
    codewords = np.ascontiguousarray(np.asarray(codewords, dtype=np.float32))
    scale = np.ascontiguousarray(np.asarray(scale, dtype=np.float32))

    nc = _get_program()
    xs = X.reshape(B, D, N)
    in_maps = [
        {"X": xs[i], "codewords": codewords, "scale": scale} for i in range(B)
    ]
    res = run_bass_kernel_spmd(nc, in_maps, core_ids=list(range(B)))
    out = np.stack([res.results[i]["E"] for i in range(B)])
    return out.astype(np.float32)
